# revision 1
# baseline (speedup 1.0000x reference)
"""Trainium2 Bass kernel for nn_Classifier_42588895707508.

Computation (see reference):
    pool_k[b, h] = max_{s < eff_k[b]} x_k[b, s, h]      (k = 1, 2)
    out[b, c]    = sum_h pool_1[b,h] W[c,h] + pool_2[b,h] W[c, 768+h] + bias[c]
where eff_k[b] is derived from the mask m_k (index of first zero; 0 -> S).

Strategy (memory-bound, ragged sequences):
  * The valid region of each sample is a contiguous DRAM prefix. Only those
    bytes ever need to touch the device (~50% of the input on average).
  * Host computes eff from the masks, then packs, per core, the transposed
    valid blocks x_k[b, :eff, :].T (h on partitions, s on the free dim)
    into one dense array P[128, 6R]: each row's segment stores its 6
    h-chunks of 128 partitions back to back (chunk-major within the slot),
    all rows concatenated along the free dim.
  * Rows (= (kind, sample) pairs, 2*512 = 1024 of them) are distributed
    round-robin by length rank across the 8 cores, so every core gets an
    identical segment-width structure -> one SPMD program, perfectly
    balanced load.
  * Segments are grouped into equal-width groups (sorted by length, padded
    by at most a few columns) so one DVE reduce_max instruction with a
    [128, 6g, w] access pattern pools g rows x 6 chunks at once.
  * The tiny linear layer runs on the tensor engine (fp32 matmul,
    K=128 per chunk accumulated in PSUM); per-row partial dot products
    are summed on the host (x1/x2 rows of one sample may land on
    different cores).
"""

import numpy as np

B, S, H, C = 512, 256, 768, 2
NCORES = 8
CH = H // 128  # 6 h-chunks of 128 partitions
KINDS = 2
SLOTS = B // NCORES  # 64 slots per kind per core
NEG = np.float32(-3.4028235e38)

# grouping cost model (slot-column units): a padded slot-column costs ~1
# (6 real columns of DMA+DVE), an extra reduce instruction only eats DVE
# slack, so it is cheap
PAD_COST = 1.0
INSTR_COST = 4.0
TILE_W = 6144  # max free width (real columns) of one SBUF data tile
RAMP = [768, 1536, 3072]  # smaller first tiles so DVE starts early
DATA_BUFS = 5


def _eff_lengths(m):
    am = np.argmin(np.asarray(m), axis=1)
    return np.where(am == 0, S, am).astype(np.int64)


def _plan_groups(widths):
    """Partition the (descending) width list into contiguous groups.

    Returns list of (start, n, gw) minimizing PAD_COST * padding +
    INSTR_COST per group, via O(n^2) DP. A group must fit in one SBUF
    data tile: n * 6 * gw <= TILE_W.
    """
    n = len(widths)
    best = np.full(n + 1, np.inf)
    best[0] = 0.0
    prev = np.zeros(n + 1, dtype=np.int64)
    for i in range(1, n + 1):
        for j in range(i - 1, -1, -1):
            if (i - j) * 6 * widths[j] > TILE_W:
                break
            pad = PAD_COST * ((i - j) * widths[j] - widths[j:i].sum())
            cost = best[j] + pad + INSTR_COST
            if cost < best[i]:
                best[i] = cost
                prev[i] = j
    groups = []
    i = n
    while i > 0:
        j = prev[i]
        groups.append((j, i - j, int(widths[j])))
        i = j
    groups.reverse()
    return groups


def _build_program(groups_all, R, tiles):
    """Build the SPMD Bass program.

    groups_all: list of (col_off, out_slot, n, w); col_off in slot-column
    units (real DRAM column = 6 * slot-column). tiles: list of
    (c0, c1, [groups...]) in real columns, group boundaries inside.
    """
    import concourse.bacc as bacc
    import concourse.mybir as mybir
    from concourse.tile import TileContext

    nc = bacc.Bacc("TRN2", target_bir_lowering=False, debug=False, num_devices=NCORES)
    p_in = nc.dram_tensor("p", [128, 6 * R], mybir.dt.float32, kind="ExternalInput")
    wt_in = nc.dram_tensor(
        "wt", [128, KINDS * CH, C], mybir.dt.float32, kind="ExternalInput"
    )
    out_d = nc.dram_tensor(
        "out", [C, 2 * SLOTS], mybir.dt.float32, kind="ExternalOutput"
    )

    with TileContext(nc) as tc:
        with (
            tc.tile_pool(name="data", bufs=DATA_BUFS) as data_pool,
            tc.tile_pool(name="pooled", bufs=1) as pooled_pool,
            tc.tile_pool(name="small", bufs=1) as small_pool,
            tc.tile_pool(name="psum", bufs=1, space="PSUM") as psum_pool,
        ):
            wt_t = small_pool.tile([128, KINDS * CH, C], mybir.dt.float32, tag="wt")
            nc.sync.dma_start(out=wt_t, in_=wt_in[:, :, :])

            # pooled[p, slot, ch]: slot = kind*64 + i, partition p = h within chunk
            pooled = pooled_pool.tile(
                [128, KINDS * SLOTS, CH], mybir.dt.float32, tag="pooled", name="pooled"
            )

            for c0, c1, tgroups in tiles:
                tw = c1 - c0
                dt = data_pool.tile([128, TILE_W], mybir.dt.float32, tag="data")
                nc.sync.dma_start(out=dt[:, :tw], in_=p_in[:, c0:c1])
                for col_off, out_slot, gn, gw in tgroups:
                    a = 6 * col_off - c0
                    view = dt[:, a : a + gn * 6 * gw].rearrange(
                        "p (g w) -> p g w", w=gw
                    )
                    nc.vector.reduce_max(
                        out=pooled[:, out_slot : out_slot + gn, :],
                        in_=view,
                        axis=mybir.AxisListType.X,
                    )

            out_sb = small_pool.tile([C, 2 * SLOTS], mybir.dt.float32, tag="osb")
            for k in range(KINDS):
                ps = psum_pool.tile([C, SLOTS], mybir.dt.float32, tag=f"ps{k}")
                for ch in range(CH):
                    nc.tensor.matmul(
                        ps,
                        lhsT=wt_t[:, k * CH + ch, :],
                        rhs=pooled[:, k * SLOTS : (k + 1) * SLOTS, ch],
                        start=(ch == 0),
                        stop=(ch == CH - 1),
                    )
                nc.scalar.copy(
                    out=out_sb[:, k * SLOTS : (k + 1) * SLOTS], in_=ps
                )
            nc.sync.dma_start(out=out_d[:, :], in_=out_sb)

    nc.compile()
    return nc


_NC_CACHE = {}


def kernel(x1, x2, m1, m2, W, b, _run_opts=None):
    from concourse.bass_utils import run_bass_kernel_spmd

    x1 = np.asarray(x1)
    x2 = np.asarray(x2)
    W = np.asarray(W, dtype=np.float32)
    b = np.asarray(b, dtype=np.float32)
    xs = (x1, x2)
    effs = [_eff_lengths(m1), _eff_lengths(m2)]
    # descending length order per kind; slot i on core c holds sample
    # orders[k][i*NCORES + c]
    orders = [np.argsort(-effs[k], kind="stable") for k in range(KINDS)]
    # slot width = max eff among the 8 cores' samples of that slot
    slot_w = [
        effs[k][orders[k][:: NCORES]].astype(np.int64) for k in range(KINDS)
    ]  # [64] each, descending

    # group slots (per kind) into equal-width reduce groups
    raw_groups = []  # (kind, start, n, gw)
    for k in range(KINDS):
        for start, n, gw in _plan_groups(slot_w[k]):
            raw_groups.append((k, start, n, gw))
    # widest groups first; the short groups at the tail keep the final
    # reduce after the last DMA tile small
    raw_groups.sort(key=lambda g: -g[3])
    emit = raw_groups
    col = 0  # slot-column units
    groups_all = []  # (col_off, out_slot, n_slots, width) in emission order
    slot_cols = [np.zeros(SLOTS, dtype=np.int64) for _ in range(KINDS)]
    slot_gw = [np.zeros(SLOTS, dtype=np.int64) for _ in range(KINDS)]
    for k, start, n, gw in emit:
        groups_all.append((col, k * SLOTS + start, n, gw))
        for j in range(n):
            slot_cols[k][start + j] = col + j * gw
            slot_gw[k][start + j] = gw
        col += n * gw
    R = col

    # DMA tiles (real columns): whole groups per tile; the first few tiles
    # are smaller (RAMP) so the vector engine starts reducing early
    tiles = []
    cur = None
    for g in groups_all:
        col_off, _, gn, gw = g
        a0, a1 = 6 * col_off, 6 * (col_off + gn * gw)
        cap = RAMP[len(tiles)] if len(tiles) < len(RAMP) else TILE_W
        if cur is not None and (a1 - cur[0]) <= cap:
            cur[1] = a1
            cur[2].append(g)
        else:
            if cur is not None:
                tiles.append(tuple(cur))
            cur = [a0, a1, [g]]
    if cur is not None:
        tiles.append(tuple(cur))
    assert all(c1 - c0 <= TILE_W for c0, c1, _ in tiles)

    # pack per-core data: P[core][p, 6*off + ch*w + j] = x_k[b, j, ch*128+p]
    packs = np.full((NCORES, 128, 6 * R), NEG, dtype=np.float32)
    for k in range(KINDS):
        xk, eff, order = xs[k], effs[k], orders[k]
        for i in range(SLOTS):
            off = slot_cols[k][i]
            w = slot_gw[k][i]
            for c in range(NCORES):
                bidx = order[i * NCORES + c]
                e = int(eff[bidx])
                dst = packs[c][:, 6 * off : 6 * (off + w)].reshape(128, 6, w)
                src = xk[bidx, :e, :].T.reshape(6, 128, e)
                dst[:, :, :e] = src.transpose(1, 0, 2)

    # weights, laid out so lhsT slices are [128 (h), C] per (kind, chunk)
    wtp = np.ascontiguousarray(
        W.reshape(C, KINDS, CH, 128).transpose(3, 1, 2, 0)
    ).reshape(128, KINDS * CH, C)

    key = (R, tuple(groups_all), tuple((c0, c1) for c0, c1, _ in tiles))
    nc = _NC_CACHE.get(key)
    if nc is None:
        nc = _build_program(groups_all, R, tiles)
        _NC_CACHE[key] = nc
    in_maps = [{"p": packs[c], "wt": wtp} for c in range(NCORES)]

    res = None
    last_err = None
    for _attempt in range(3):
        try:
            res = run_bass_kernel_spmd(
                nc, in_maps, core_ids=list(range(NCORES)), **(_run_opts or {})
            )
            break
        except Exception as e:  # wedged device etc. -- retry
            last_err = e
    if res is None:
        raise last_err

    # combine per-row partials
    out_full = np.zeros((B, C), dtype=np.float32)
    res_all = np.stack([res.results[c]["out"] for c in range(NCORES)])  # [8, C, 128]
    for k in range(KINDS):
        part = res_all[:, :, k * SLOTS : (k + 1) * SLOTS]  # [core, C, slot]
        part = part.transpose(2, 0, 1).reshape(B, C)  # [(slot, core), C]
        out_full[orders[k]] += part
    out_full += b[None, :]
    if _run_opts is not None:
        kernel._last_res = res
    return out_full



# revision 2
# speedup vs baseline: 1.1718x; 1.1718x over previous
"""Trainium2 Bass kernel for nn_Classifier_42588895707508 (fp16 fold version).

Computation (see reference):
    pool_k[b, h] = max_{s < eff_k[b]} x_k[b, s, h]      (k = 1, 2)
    out[b, c]    = sum_h pool_1[b,h] W[c,h] + pool_2[b,h] W[c, 768+h] + bias[c]
where eff_k[b] is derived from the mask m_k (index of first zero; 0 -> S).

Strategy (memory-bound, ragged sequences):
  * Only the valid prefix of each sample row touches the device, packed
    densely per core (transposed: h%128 on partitions, per slot 6 h-chunks
    x width contiguous), in FP16 -- halving HBM traffic vs fp32. The 2e-2
    harness gate leaves ~100x margin for fp16 rounding.
  * Rows (kind, sample) are distributed round-robin by length rank across
    the 8 cores -> identical SPMD program, balanced load.
  * Pooling runs on the DVE as tensor-tensor max folds
    (scalar_tensor_tensor with op0=mult(1.0), op1=max), which in 16-bit
    get the 2x_1p perf mode (2 elem/lane/cyc) -- unlike tensor_reduce
    which is stuck at 1x. Each fold halves a group's width with an
    overlapped (idempotent, max) split so every level keeps 4B alignment
    with only pad-to-multiple-of-4.
  * Folds stop at width 8 writing into a persistent staging tile
    [128, 128slots*6ch, 8]; three shared folds (8->4->2->1) finish all
    slots of a kind at once, then the tiny linear layer runs on the
    tensor engine in fp16 (K=128 per chunk, PSUM f32).
  * Groups (equal padded width) are planned by a small DP balancing DMA
    padding vs DVE instruction count; DMA tiles ramp up at the start and
    taper at the end so the vector engine starts early and drains fast.
"""

import numpy as np

B, S, H, C = 512, 256, 768, 2
NCORES = 8
CH = H // 128            # 6 h-chunks of 128 partitions
KINDS = 2
SLOTS = B // NCORES      # 64 slots per kind per core
SLOT_TOT = KINDS * SLOTS
NEG = np.float16(-60000.0)

TILE_W = 16384           # fp16 elems per partition per SBUF data tile
DATA_BUFS = 4
RAMP = [1024, 2048, 4096, 8192]

# planning cost model
DMA_NS_PER_COL = 1536.0 / 385.0   # one slot-col = 6 elems x 128 part x 2B
DVE_GHZ = 0.96
OP_OVERHEAD_CYC = 58.0
LAMBDAS = [0.0, 0.25, 0.5, 1.0, 2.0, 4.0, 8.0]


def _eff_lengths(m):
    am = np.argmin(np.asarray(m), axis=1)
    return np.where(am == 0, S, am).astype(np.int64)


def _ceil4(x):
    return (int(x) + 3) // 4 * 4


def _chain(W):
    """Fold chain for a group of width W (mult of 4, >= 8).

    Returns (n_ops, out_elems_per_row): in-place halving folds down to
    <= 16 wide, then one fold to width 8 (into staging)."""
    ops, outs, c = 0, 0, int(W)
    while c > 16:
        h = _ceil4((c + 1) // 2)
        ops += 1
        outs += h
        c = h
    ops += 1
    outs += 8
    return ops, outs


def _plan_widths(widths, lam):
    """Group descending padded widths into equal-width groups via DP.

    Cost per group: DMA of n*W slot-cols + lam * per-op overhead.
    Constraint: n*CH*W <= TILE_W. Returns final width per slot."""
    n = len(widths)
    INF = float("inf")
    best = [INF] * (n + 1)
    best[0] = 0.0
    prev = [0] * (n + 1)
    for i in range(1, n + 1):
        j = i - 1
        while j >= 0:
            Wg = widths[j]
            if (i - j) * CH * Wg > TILE_W:
                break
            ops, _ = _chain(Wg)
            cost = (
                best[j]
                + (i - j) * Wg * DMA_NS_PER_COL
                + lam * ops * OP_OVERHEAD_CYC / DVE_GHZ
            )
            if cost < best[i]:
                best[i] = cost
                prev[i] = j
            j -= 1
    out = np.zeros(n, dtype=np.int64)
    i = n
    while i > 0:
        j = prev[i]
        out[j:i] = widths[j]
        i = j
    return out


def _estimate(final_widths_by_kind):
    """(dma_ns, dve_ns) rough estimates for a plan."""
    cols = 0
    work_cyc = 0.0
    ops = 0
    for fw in final_widths_by_kind:
        i = 0
        n = len(fw)
        while i < n:
            j = i
            while j < n and fw[j] == fw[i] and (j - i + 1) * CH * fw[i] <= TILE_W:
                j += 1
            g = j - i
            c_ops, c_outs = _chain(fw[i])
            cols += g * fw[i]
            work_cyc += CH * g * c_outs / 2.0
            ops += c_ops
            i = j
    dma_ns = cols * DMA_NS_PER_COL + 3000.0
    dve_ns = (work_cyc + ops * OP_OVERHEAD_CYC + 4000.0) / DVE_GHZ
    return dma_ns, dve_ns


def _make_plan(slot_w):
    """slot_w: [kind][64] true widths desc. Returns final padded widths."""
    padded = [np.maximum([_ceil4(w) for w in sw], 8) for sw in slot_w]
    best = None
    for lam in LAMBDAS:
        fw = [_plan_widths(p, lam) for p in padded]
        dma, dve = _estimate(fw)
        obj = (max(dma, dve), dma)
        if best is None or obj < best[0]:
            best = (obj, fw)
    return best[1]


def _layout(final_w):
    """Assign slot offsets / groups / tiles from final widths.

    Returns (R, tiles) with tiles = [(c0, c1, [(a, s0, g, W), ...])],
    offsets in fp16 elems per partition; s0 = global slot index k*64+i.
    """
    stream = []  # (kind, slot, W)
    for k in range(KINDS):
        for i in range(SLOTS):
            stream.append((k, i, int(final_w[k][i])))
    total = sum(CH * W for _, _, W in stream)

    offs = [np.zeros(SLOTS, dtype=np.int64) for _ in range(KINDS)]
    tiles = []
    cur_groups = []  # per tile: list of [a, s0, g, W]
    cur_start = 0
    cur_elems = 0
    off = 0
    done = 0

    def close_tile():
        nonlocal cur_groups, cur_start, cur_elems
        if cur_elems:
            tiles.append((cur_start, cur_start + cur_elems, cur_groups))
        cur_groups = []
        cur_start = off
        cur_elems = 0

    for (k, i, W) in stream:
        sz = CH * W
        rem = total - done
        cap = RAMP[len(tiles)] if len(tiles) < len(RAMP) else TILE_W
        if cap == TILE_W and rem <= TILE_W + TILE_W // 2 and rem > 6144:
            cap = min(cap, max(4096, (rem * 2) // 3))
        if cur_elems and cur_elems + sz > cap:
            close_tile()
        s0 = k * SLOTS + i
        if (
            cur_groups
            and cur_groups[-1][3] == W
            and cur_groups[-1][1] + cur_groups[-1][2] == s0
        ):
            cur_groups[-1][2] += 1
        else:
            cur_groups.append([off, s0, 1, W])
        offs[k][i] = off
        off += sz
        cur_elems += sz
        done += sz
    close_tile()
    tiles = [(c0, c1, tuple(tuple(g) for g in gs)) for c0, c1, gs in tiles]
    return off, tiles, offs


def _build_program(R, tiles):
    import concourse.bacc as bacc
    import concourse.mybir as mybir
    from concourse.tile import TileContext

    f16 = mybir.dt.float16
    f32 = mybir.dt.float32
    MULT = mybir.AluOpType.mult
    MAX = mybir.AluOpType.max

    # last tile index touching each kind, for epilogue placement
    last_tile_of_kind = [0, 0]
    for ti, (_, _, groups) in enumerate(tiles):
        for (_, s0, _, _) in groups:
            last_tile_of_kind[s0 // SLOTS] = ti

    nc = bacc.Bacc("TRN2", target_bir_lowering=False, debug=False, num_devices=NCORES)
    p_in = nc.dram_tensor("p", [128, R], f16, kind="ExternalInput")
    wt_in = nc.dram_tensor("wt", [128, KINDS * CH, C], f16, kind="ExternalInput")
    out_d = nc.dram_tensor("out", [C, SLOT_TOT], f32, kind="ExternalOutput")

    with TileContext(nc) as tc:
        with (
            tc.tile_pool(name="data", bufs=DATA_BUFS) as data_pool,
            tc.tile_pool(name="stage", bufs=1) as stage_pool,
            tc.tile_pool(name="small", bufs=1) as small_pool,
            tc.tile_pool(name="psum", bufs=1, space="PSUM") as psum_pool,
        ):
            wt_t = small_pool.tile([128, KINDS * CH, C], f16, tag="wt")
            nc.sync.dma_start(out=wt_t, in_=wt_in[:, :, :])
            stage = stage_pool.tile([128, SLOT_TOT * CH, 8], f16, tag="stage")
            out_sb = small_pool.tile([C, SLOT_TOT], f32, tag="osb")

            def fold(view, w, h, out_view=None):
                # out = max(view[:, :, :h], view[:, :, w-h:w]) (overlap ok)
                nc.vector.scalar_tensor_tensor(
                    out=view[:, :, :h] if out_view is None else out_view,
                    in0=view[:, :, :h],
                    scalar=1.0,
                    in1=view[:, :, w - h : w],
                    op0=MULT,
                    op1=MAX,
                )

            def epilogue(k):
                rows = stage[:, k * SLOTS * CH : (k + 1) * SLOTS * CH, :]
                for (w, h) in ((8, 4), (4, 2), (2, 1)):
                    fold(rows, w, h)
                srg = stage[:, :, :].rearrange("p (s c) e -> p s c e", c=CH)
                ps = psum_pool.tile([C, SLOTS], f32, tag=f"ps{k}")
                for ch in range(CH):
                    nc.tensor.matmul(
                        ps,
                        lhsT=wt_t[:, k * CH + ch, :],
                        rhs=srg[:, k * SLOTS : (k + 1) * SLOTS, ch, 0],
                        start=(ch == 0),
                        stop=(ch == CH - 1),
                    )
                nc.scalar.copy(out=out_sb[:, k * SLOTS : (k + 1) * SLOTS], in_=ps)

            for ti, (c0, c1, groups) in enumerate(tiles):
                tw = c1 - c0
                dt = data_pool.tile([128, TILE_W], f16, tag="data")
                nc.sync.dma_start(out=dt[:, :tw], in_=p_in[:, c0:c1])
                for (a, s0, gn, W) in groups:
                    base = a - c0
                    view = dt[:, base : base + gn * CH * W].rearrange(
                        "p (r w) -> p r w", w=W
                    )
                    w = W
                    while w > 16:
                        h = _ceil4((w + 1) // 2)
                        fold(view, w, h)
                        w = h
                    st = stage[:, s0 * CH : (s0 + gn) * CH, :]
                    fold(view, w, 8, out_view=st)
                for k in range(KINDS):
                    if last_tile_of_kind[k] == ti:
                        epilogue(k)

            nc.sync.dma_start(out=out_d[:, :], in_=out_sb)

    nc.compile()
    return nc


_NC_CACHE = {}


def kernel(x1, x2, m1, m2, W, b, _run_opts=None):
    from concourse.bass_utils import run_bass_kernel_spmd

    x1 = np.asarray(x1)
    x2 = np.asarray(x2)
    W32 = np.asarray(W, dtype=np.float32)
    b32 = np.asarray(b, dtype=np.float32)
    effs = [_eff_lengths(m1), _eff_lengths(m2)]
    orders = [np.argsort(-effs[k], kind="stable") for k in range(KINDS)]
    slot_w = [effs[k][orders[k][:: NCORES]].astype(np.int64) for k in range(KINDS)]

    final_w = _make_plan(slot_w)
    R, tiles, offs = _layout(final_w)

    key = (R, tuple(tiles))
    nc = _NC_CACHE.get(key)
    if nc is None:
        nc = _build_program(R, tiles)
        _NC_CACHE[key] = nc

    # pack per-core fp16 data
    xh = [x1.astype(np.float16), x2.astype(np.float16)]
    packs = np.full((NCORES, 128, R), NEG, dtype=np.float16)
    for k in range(KINDS):
        eff, order = effs[k], orders[k]
        xk = xh[k]
        for i in range(SLOTS):
            off = int(offs[k][i])
            v = int(final_w[k][i])
            for c in range(NCORES):
                bidx = order[i * NCORES + c]
                e = int(eff[bidx])
                dst = packs[c][:, off : off + CH * v].reshape(128, CH, v)
                dst[:, :, :e] = xk[bidx, :e, :].reshape(e, CH, 128).transpose(2, 1, 0)

    wtp = np.ascontiguousarray(
        W32.astype(np.float16).reshape(C, KINDS, CH, 128).transpose(3, 1, 2, 0)
    ).reshape(128, KINDS * CH, C)

    in_maps = [{"p": packs[c], "wt": wtp} for c in range(NCORES)]

    res = None
    last_err = None
    for _attempt in range(3):
        try:
            res = run_bass_kernel_spmd(
                nc, in_maps, core_ids=list(range(NCORES)), **(_run_opts or {})
            )
            break
        except Exception as e:  # wedged device etc. -- retry
            last_err = e
    if res is None:
        raise last_err

    out_full = np.zeros((B, C), dtype=np.float32)
    res_all = np.stack([res.results[c]["out"] for c in range(NCORES)])  # [8, C, 128]
    for k in range(KINDS):
        part = res_all[:, :, k * SLOTS : (k + 1) * SLOTS]  # [core, C, slot]
        part = part.transpose(2, 0, 1).reshape(B, C)  # [(slot, core), C]
        out_full[orders[k]] += part
    out_full += b32[None, :]
    if _run_opts is not None:
        kernel._last_res = res
    return out_full


# revision 3
# speedup vs baseline: 1.6806x; 1.4342x over previous
"""Trainium2 Bass kernel for nn_Classifier_42588895707508 (fp16 fold version).

Computation (see reference):
    pool_k[b, h] = max_{s < eff_k[b]} x_k[b, s, h]      (k = 1, 2)
    out[b, c]    = sum_h pool_1[b,h] W[c,h] + pool_2[b,h] W[c, 768+h] + bias[c]
where eff_k[b] is derived from the mask m_k (index of first zero; 0 -> S).

Strategy (memory-bound, ragged sequences):
  * Only the valid prefix of each sample row touches the device, packed
    densely per core (transposed: h%128 on partitions, per slot 6 h-chunks
    x width contiguous), in FP16 -- halving HBM traffic vs fp32. The 2e-2
    harness gate leaves ~100x margin for fp16 rounding.
  * Rows (kind, sample) are distributed round-robin by length rank across
    the 8 cores -> identical SPMD program, balanced load.
  * Pooling runs on the DVE as tensor-tensor max folds
    (tensor_tensor with op=max), which in 16-bit
    get the 2x_1p perf mode (2 elem/lane/cyc) -- unlike tensor_reduce
    which is stuck at 1x. Each fold halves a group's width with an
    overlapped (idempotent, max) split so every level keeps 4B alignment
    with only pad-to-multiple-of-4.
  * Folds stop at width 8 writing into a persistent staging tile
    [128, 128slots*6ch, 8]; three shared folds (8->4->2->1) finish all
    slots of a kind at once, then the tiny linear layer runs on the
    tensor engine in fp16 (K=128 per chunk, PSUM f32).
  * Groups (equal padded width) are planned by a small DP balancing DMA
    padding vs DVE instruction count; DMA tiles ramp up at the start and
    taper at the end so the vector engine starts early and drains fast.
"""

import numpy as np

B, S, H, C = 512, 256, 768, 2
NCORES = 8
CH = H // 128            # 6 h-chunks of 128 partitions
KINDS = 2
SLOTS = B // NCORES      # 64 slots per kind per core
SLOT_TOT = KINDS * SLOTS
NEG = np.float16(-60000.0)

TILE_W = 16384           # fp16 elems per partition per SBUF data tile
DATA_BUFS = 4
RAMP = [1024, 2048, 4096, 8192]

# planning cost model
DMA_NS_PER_COL = 1536.0 / 385.0   # one slot-col = 6 elems x 128 part x 2B
DVE_GHZ = 0.96
OP_OVERHEAD_CYC = 58.0
LAMBDAS = [0.0, 0.25, 0.5, 1.0, 2.0, 4.0, 8.0]


def _eff_lengths(m):
    am = np.argmin(np.asarray(m), axis=1)
    return np.where(am == 0, S, am).astype(np.int64)


def _ceil4(x):
    return (int(x) + 3) // 4 * 4


def _chain(W):
    """Fold chain for a group of width W (mult of 4, >= 8).

    Returns (n_ops, out_elems_per_row): in-place halving folds down to
    <= 16 wide, then one fold to width 8 (into staging)."""
    ops, outs, c = 0, 0, int(W)
    while c > 16:
        h = _ceil4((c + 1) // 2)
        ops += 1
        outs += h
        c = h
    ops += 1
    outs += 8
    return ops, outs


def _plan_widths(widths, lam):
    """Group descending padded widths into equal-width groups via DP.

    Cost per group: DMA of n*W slot-cols + lam * per-op overhead.
    Constraint: n*CH*W <= TILE_W. Returns final width per slot."""
    n = len(widths)
    INF = float("inf")
    best = [INF] * (n + 1)
    best[0] = 0.0
    prev = [0] * (n + 1)
    for i in range(1, n + 1):
        j = i - 1
        while j >= 0:
            Wg = widths[j]
            if (i - j) * CH * Wg > TILE_W:
                break
            ops, _ = _chain(Wg)
            cost = (
                best[j]
                + (i - j) * Wg * DMA_NS_PER_COL
                + lam * ops * OP_OVERHEAD_CYC / DVE_GHZ
            )
            if cost < best[i]:
                best[i] = cost
                prev[i] = j
            j -= 1
    out = np.zeros(n, dtype=np.int64)
    i = n
    while i > 0:
        j = prev[i]
        out[j:i] = widths[j]
        i = j
    return out


def _estimate(final_widths_by_kind):
    """(dma_ns, dve_ns) rough estimates for a plan."""
    cols = 0
    work_cyc = 0.0
    ops = 0
    for fw in final_widths_by_kind:
        i = 0
        n = len(fw)
        while i < n:
            j = i
            while j < n and fw[j] == fw[i] and (j - i + 1) * CH * fw[i] <= TILE_W:
                j += 1
            g = j - i
            c_ops, c_outs = _chain(fw[i])
            cols += g * fw[i]
            work_cyc += CH * g * c_outs / 2.0
            ops += c_ops
            i = j
    dma_ns = cols * DMA_NS_PER_COL + 3000.0
    dve_ns = (work_cyc + ops * OP_OVERHEAD_CYC + 4000.0) / DVE_GHZ
    return dma_ns, dve_ns


def _make_plan(slot_w):
    """slot_w: [kind][64] true widths desc. Returns final padded widths."""
    padded = [np.maximum([_ceil4(w) for w in sw], 8) for sw in slot_w]
    best = None
    for lam in LAMBDAS:
        fw = [_plan_widths(p, lam) for p in padded]
        dma, dve = _estimate(fw)
        obj = (max(dma, dve), dma)
        if best is None or obj < best[0]:
            best = (obj, fw)
    return best[1]


def _layout(final_w):
    """Assign slot offsets / groups / tiles from final widths.

    Returns (R, tiles) with tiles = [(c0, c1, [(a, s0, g, W), ...])],
    offsets in fp16 elems per partition; s0 = global slot index k*64+i.
    """
    stream = []  # (kind, slot, W)
    for k in range(KINDS):
        for i in range(SLOTS):
            stream.append((k, i, int(final_w[k][i])))
    total = sum(CH * W for _, _, W in stream)

    offs = [np.zeros(SLOTS, dtype=np.int64) for _ in range(KINDS)]
    tiles = []
    cur_groups = []  # per tile: list of [a, s0, g, W]
    cur_start = 0
    cur_elems = 0
    off = 0
    done = 0

    def close_tile():
        nonlocal cur_groups, cur_start, cur_elems
        if cur_elems:
            tiles.append((cur_start, cur_start + cur_elems, cur_groups))
        cur_groups = []
        cur_start = off
        cur_elems = 0

    for (k, i, W) in stream:
        sz = CH * W
        rem = total - done
        cap = RAMP[len(tiles)] if len(tiles) < len(RAMP) else TILE_W
        if cap == TILE_W and rem <= TILE_W + TILE_W // 2 and rem > 6144:
            cap = min(cap, max(4096, (rem * 2) // 3))
        if cur_elems and cur_elems + sz > cap:
            close_tile()
        s0 = k * SLOTS + i
        if (
            cur_groups
            and cur_groups[-1][3] == W
            and cur_groups[-1][1] + cur_groups[-1][2] == s0
        ):
            cur_groups[-1][2] += 1
        else:
            cur_groups.append([off, s0, 1, W])
        offs[k][i] = off
        off += sz
        cur_elems += sz
        done += sz
    close_tile()
    tiles = [(c0, c1, tuple(tuple(g) for g in gs)) for c0, c1, gs in tiles]
    return off, tiles, offs


def _build_program(R, tiles):
    import concourse.bacc as bacc
    import concourse.mybir as mybir
    from concourse.tile import TileContext

    f16 = mybir.dt.float16
    f32 = mybir.dt.float32
    MAX = mybir.AluOpType.max

    # last tile index touching each kind, for epilogue placement
    last_tile_of_kind = [0, 0]
    for ti, (_, _, groups) in enumerate(tiles):
        for (_, s0, _, _) in groups:
            last_tile_of_kind[s0 // SLOTS] = ti

    nc = bacc.Bacc("TRN2", target_bir_lowering=False, debug=False, num_devices=NCORES)
    p_in = nc.dram_tensor("p", [128, R], f16, kind="ExternalInput")
    wt_in = nc.dram_tensor("wt", [128, KINDS * CH, C], f16, kind="ExternalInput")
    out_d = nc.dram_tensor("out", [C, SLOT_TOT], f32, kind="ExternalOutput")

    with TileContext(nc) as tc:
        with (
            tc.tile_pool(name="data", bufs=DATA_BUFS) as data_pool,
            tc.tile_pool(name="stage", bufs=1) as stage_pool,
            tc.tile_pool(name="small", bufs=1) as small_pool,
            tc.tile_pool(name="psum", bufs=1, space="PSUM") as psum_pool,
        ):
            wt_t = small_pool.tile([128, KINDS * CH, C], f16, tag="wt")
            nc.sync.dma_start(out=wt_t, in_=wt_in[:, :, :])
            stage = stage_pool.tile([128, SLOT_TOT * CH, 8], f16, tag="stage")
            out_sb = small_pool.tile([C, SLOT_TOT], f32, tag="osb")

            def fold(view, w, h, out_view=None):
                # out = max(view[:, :, :h], view[:, :, w-h:w]) (overlap ok)
                nc.vector.tensor_tensor(
                    out=view[:, :, :h] if out_view is None else out_view,
                    in0=view[:, :, :h],
                    in1=view[:, :, w - h : w],
                    op=MAX,
                )

            def epilogue(k):
                rows = stage[:, k * SLOTS * CH : (k + 1) * SLOTS * CH, :]
                for (w, h) in ((8, 4), (4, 2), (2, 1)):
                    fold(rows, w, h)
                srg = stage[:, :, :].rearrange("p (s c) e -> p s c e", c=CH)
                ps = psum_pool.tile([C, SLOTS], f32, tag=f"ps{k}")
                for ch in range(CH):
                    nc.tensor.matmul(
                        ps,
                        lhsT=wt_t[:, k * CH + ch, :],
                        rhs=srg[:, k * SLOTS : (k + 1) * SLOTS, ch, 0],
                        start=(ch == 0),
                        stop=(ch == CH - 1),
                    )
                nc.scalar.copy(out=out_sb[:, k * SLOTS : (k + 1) * SLOTS], in_=ps)

            for ti, (c0, c1, groups) in enumerate(tiles):
                tw = c1 - c0
                dt = data_pool.tile([128, TILE_W], f16, tag="data")
                nc.sync.dma_start(out=dt[:, :tw], in_=p_in[:, c0:c1])
                for (a, s0, gn, W) in groups:
                    base = a - c0
                    view = dt[:, base : base + gn * CH * W].rearrange(
                        "p (r w) -> p r w", w=W
                    )
                    w = W
                    while w > 16:
                        h = _ceil4((w + 1) // 2)
                        fold(view, w, h)
                        w = h
                    st = stage[:, s0 * CH : (s0 + gn) * CH, :]
                    fold(view, w, 8, out_view=st)
                for k in range(KINDS):
                    if last_tile_of_kind[k] == ti:
                        epilogue(k)

            nc.sync.dma_start(out=out_d[:, :], in_=out_sb)

    nc.compile()
    return nc


_NC_CACHE = {}


def kernel(x1, x2, m1, m2, W, b, _run_opts=None):
    from concourse.bass_utils import run_bass_kernel_spmd

    x1 = np.asarray(x1)
    x2 = np.asarray(x2)
    W32 = np.asarray(W, dtype=np.float32)
    b32 = np.asarray(b, dtype=np.float32)
    effs = [_eff_lengths(m1), _eff_lengths(m2)]
    orders = [np.argsort(-effs[k], kind="stable") for k in range(KINDS)]
    slot_w = [effs[k][orders[k][:: NCORES]].astype(np.int64) for k in range(KINDS)]

    final_w = _make_plan(slot_w)
    R, tiles, offs = _layout(final_w)

    key = (R, tuple(tiles))
    nc = _NC_CACHE.get(key)
    if nc is None:
        nc = _build_program(R, tiles)
        _NC_CACHE[key] = nc

    # pack per-core fp16 data
    xh = [x1.astype(np.float16), x2.astype(np.float16)]
    packs = np.full((NCORES, 128, R), NEG, dtype=np.float16)
    for k in range(KINDS):
        eff, order = effs[k], orders[k]
        xk = xh[k]
        for i in range(SLOTS):
            off = int(offs[k][i])
            v = int(final_w[k][i])
            for c in range(NCORES):
                bidx = order[i * NCORES + c]
                e = int(eff[bidx])
                dst = packs[c][:, off : off + CH * v].reshape(128, CH, v)
                dst[:, :, :e] = xk[bidx, :e, :].reshape(e, CH, 128).transpose(2, 1, 0)

    wtp = np.ascontiguousarray(
        W32.astype(np.float16).reshape(C, KINDS, CH, 128).transpose(3, 1, 2, 0)
    ).reshape(128, KINDS * CH, C)

    in_maps = [{"p": packs[c], "wt": wtp} for c in range(NCORES)]

    res = None
    last_err = None
    for _attempt in range(3):
        try:
            res = run_bass_kernel_spmd(
                nc, in_maps, core_ids=list(range(NCORES)), **(_run_opts or {})
            )
            break
        except Exception as e:  # wedged device etc. -- retry
            last_err = e
    if res is None:
        raise last_err

    out_full = np.zeros((B, C), dtype=np.float32)
    res_all = np.stack([res.results[c]["out"] for c in range(NCORES)])  # [8, C, 128]
    for k in range(KINDS):
        part = res_all[:, :, k * SLOTS : (k + 1) * SLOTS]  # [core, C, slot]
        part = part.transpose(2, 0, 1).reshape(B, C)  # [(slot, core), C]
        out_full[orders[k]] += part
    out_full += b32[None, :]
    if _run_opts is not None:
        kernel._last_res = res
    return out_full
